# revision 18
# baseline (speedup 1.0000x reference)
"""Trainium2 Bass kernel for MViTv2-style attention (decomposed rel-pos bias).

Problem: B=8, H=W=32, DIM=768, NH=12, HD=64, S=1024.
Sharding: data-parallel, one batch element per NeuronCore (8 cores).

Per-core pipeline (all matmuls float32r, 1 cyc/row):
  1. qkT = wqk.T @ x.T  (transposed projection; q pre-scaled by 1/sqrt(hd))
  2. v   = x @ wv.T     (natural layout, bias folded into proj bias on host)
  3. rel_h/rel_w terms via per-row-group matmuls into the augmented q rows
  4. attnT[sk,sq] = k'.T @ q' with K=128 augmentation:
       q' = [q*scale; rel_h; rel_w],  k' = [k; onehot_h; onehot_w]
     -> QK^T + decomposed rel-pos bias in ONE matmul
  5. exp on ScalarE (no max-sub; logits are O(1)); PV with ones-augmented v
     -> softmax denominator appears as out row 64 for free
  6. reciprocal via exp(-ln(d)); K=1 ones-matmul broadcasts it across
     partitions; DVE multiply normalizes
  7. proj matmul + bias; output transposed (host un-transposes)
"""
import numpy as np

B, H, W, DIM, NH = 8, 32, 32, 768, 12
HD = DIM // NH          # 64
S = H * W               # 1024
SCALE = HD ** -0.5
NCORES = 8

PE_INC_PV = [4, 6, 8, 10, 12, 14, 15, 16]  # s_pe offset of PV(t) within an iter


def build_nc():
    import concourse.bass as bass
    import concourse.mybir as mybir
    from contextlib import ExitStack

    F32 = mybir.dt.float32
    F32R = mybir.dt.float32r
    AF = mybir.ActivationFunctionType

    nc = bass.Bass(detect_race_conditions=False)

    # ---- DRAM parameters (per core) ----
    xT_e = nc.declare_dram_parameter("xT", [DIM, S], F32R, isOutput=False)
    wqk_e = nc.declare_dram_parameter("wqk", [DIM, 2 * DIM], F32R, isOutput=False)
    wv_e = nc.declare_dram_parameter("wv", [DIM, DIM], F32R, isOutput=False)
    wproj_e = nc.declare_dram_parameter("wproj", [DIM, DIM], F32R, isOutput=False)
    relh_e = nc.declare_dram_parameter("relh", [HD, H * H], F32R, isOutput=False)
    relw_e = nc.declare_dram_parameter("relw", [HD, W * W], F32R, isOutput=False)
    oneh_e = nc.declare_dram_parameter("onehot", [HD, S], F32R, isOutput=False)
    onescol_e = nc.declare_dram_parameter("onescol", [128, NH], F32R, isOutput=False)
    ones64_e = nc.declare_dram_parameter("ones64", [1, HD], F32R, isOutput=False)
    qkb_e = nc.declare_dram_parameter("qkb", [128, 12], F32, isOutput=False)
    projb_e = nc.declare_dram_parameter("projb", [128, 6], F32, isOutput=False)
    outT_e = nc.declare_dram_parameter("outT", [DIM, S], F32, isOutput=True)

    ctx = ExitStack()
    with ctx:
        # ---- persistent SBUF ----
        qaug = ctx.enter_context(nc.sbuf_tensor("qaug", [128, NH, S], F32R))
        kaug = ctx.enter_context(nc.sbuf_tensor("kaug", [128, NH, S], F32R))
        vaug = ctx.enter_context(nc.sbuf_tensor("vaug", [128, 8, NH * 65], F32R))
        scr = [ctx.enter_context(nc.sbuf_tensor(f"scr{i}", [128, 512], F32R))
               for i in range(2)]
        bcast_sb = ctx.enter_context(nc.sbuf_tensor("bcast_sb", [64, 512], F32))
        ln_sb = bcast_sb[0:1, :]
        recip_sb = ctx.enter_context(nc.sbuf_tensor("recip_sb", [1, 512], F32R))
        ones64 = ctx.enter_context(nc.sbuf_tensor("ones64_sb", [1, HD], F32R))
        qkb_sb = ctx.enter_context(nc.sbuf_tensor("qkb_sb", [128, 12], F32))
        projb_sb = ctx.enter_context(nc.sbuf_tensor("projb_sb", [128, 6], F32))

        # ---- PSUM (8 banks) ----
        qk_ps = [ctx.enter_context(nc.psum_tensor(f"qk_ps{i}", [128, 512], F32))
                 for i in range(2)]
        at_ps = [ctx.enter_context(nc.psum_tensor(f"at_ps{i}", [128, 512], F32))
                 for i in range(2)]
        out_ps = [ctx.enter_context(nc.psum_tensor(f"out_ps{i}", [128, 512], F32))
                  for i in range(2)]
        bc_ps = ctx.enter_context(nc.psum_tensor("bc_ps", [64, 512], F32))
        rel_ps = ctx.enter_context(nc.psum_tensor("rel_ps", [128, 384], F32))

        # DMA-completion sems are per-group/slot: HWDGE completions are not
        # FIFO across queues, so every wait must be an all-of-group total.
        s_l1 = ctx.enter_context(nc.semaphore("s_l1"))  # xT + wq loads
        s_l2 = ctx.enter_context(nc.semaphore("s_l2"))  # wk loads
        s_l3 = ctx.enter_context(nc.semaphore("s_l3"))  # small consts
        s_l4 = ctx.enter_context(nc.semaphore("s_l4"))  # onehot
        s_l5 = ctx.enter_context(nc.semaphore("s_l5"))  # onescol
        s_l6 = ctx.enter_context(nc.semaphore("s_l6"))  # wv loads
        s_l7 = ctx.enter_context(nc.semaphore("s_l7"))  # wproj loads
        s_cr0 = ctx.enter_context(nc.semaphore("s_cr0"))  # even-g crosses
        s_cr1 = ctx.enter_context(nc.semaphore("s_cr1"))  # odd-g crosses
        s_rh = ctx.enter_context(nc.semaphore("s_rh"))    # rel-h DMAs
        s_rw = ctx.enter_context(nc.semaphore("s_rw"))    # rel-w shift DMAs
        s_od0 = ctx.enter_context(nc.semaphore("s_od0"))  # outdT DMAs slot 0
        s_od1 = ctx.enter_context(nc.semaphore("s_od1"))  # outdT DMAs slot 1
        s_out0 = ctx.enter_context(nc.semaphore("s_out0"))  # final out even g
        s_out1 = ctx.enter_context(nc.semaphore("s_out1"))  # final out odd g
        s_pe = ctx.enter_context(nc.semaphore("s_pe"))
        s_act = ctx.enter_context(nc.semaphore("s_act"))
        s_dve = ctx.enter_context(nc.semaphore("s_dve"))

        # python-side cumulative counters
        C = {"dma": 0, "pe": 0, "act": 0, "dve": 0, "dmo": 0}

        block = ctx.enter_context(nc.Block())

        # ================= PHASE 1: projections =================
        p1 = ExitStack()
        with p1:
            xT = p1.enter_context(nc.sbuf_tensor("xT_sb", [128, 6, S], F32R))
            wA = p1.enter_context(nc.sbuf_tensor("wA", [128, 6, DIM], F32R))
            wB = p1.enter_context(nc.sbuf_tensor("wB", [128, 6, DIM], F32R))
            relh = p1.enter_context(nc.sbuf_tensor("relh_sb", [HD, H * H], F32R))
            relw = p1.enter_context(nc.sbuf_tensor("relw_sb", [HD, W * W], F32R))

            # ---- loads ----
            def _loads(sync):
                for dt in range(6):
                    sync.dma_start(out=xT[:, dt, :],
                                   in_=xT_e[dt * 128:(dt + 1) * 128, :]
                                   ).then_inc(s_l1, 16)
                for dt in range(6):
                    sync.dma_start(out=wA[:, dt, :],
                                   in_=wqk_e[dt * 128:(dt + 1) * 128, 0:DIM]
                                   ).then_inc(s_l1, 16)
                for dt in range(6):
                    sync.dma_start(out=wB[:, dt, :],
                                   in_=wqk_e[dt * 128:(dt + 1) * 128, DIM:2 * DIM]
                                   ).then_inc(s_l2, 16)
                sync.dma_start(out=relh[:], in_=relh_e[:]).then_inc(s_l3, 16)
                sync.dma_start(out=relw[:], in_=relw_e[:]).then_inc(s_l3, 16)
                sync.dma_start(out=ones64[:], in_=ones64_e[:]).then_inc(s_l3, 16)
                sync.dma_start(out=qkb_sb[:], in_=qkb_e[:]).then_inc(s_l3, 16)
                sync.dma_start(out=projb_sb[:], in_=projb_e[:]).then_inc(s_l3, 16)
                # onehot rows into kaug[64:128] for each head
                for m in range(NH):
                    sync.dma_start(out=kaug[64:128, m, :], in_=oneh_e[:]
                                   ).then_inc(s_l4, 16)
                # ones column into vaug (col 64 of each head block)
                va = vaug[:].rearrange("p t (m c) -> p t m c", c=65)
                with nc.allow_non_contiguous_dma(reason="12-elem ones cols"):
                    for sk in range(8):
                        sync.dma_start(out=va[:, sk, :, 64:65],
                                       in_=onescol_e[:].unsqueeze(2)
                                       ).then_inc(s_l5, 16)

            marks = {}
            block.sync(_loads)

            # ---- PE: q, k matmuls (pair tiles), then v, then rel ----
            qk_done = {}      # (which, jt, b) -> s_pe value
            v_done = {}
            rel_done = {}

            def _pe1(tensor):
                tensor.wait_ge(s_l1, 12 * 16)
                first_k = True
                for which, wsb in (("q", wA), ("k", wB)):
                    if which == "k":
                        pass  # wB load mark checked below
                    for jt in range(6):
                        for b in range(2):
                            idx = jt * 2 + b
                            if which == "k" and first_k:
                                tensor.wait_ge(s_l2, 6 * 16)
                                first_k = False
                            # psum WAR: bank reused by group idx-2
                            key = (which, jt, b)
                            prev = qk_war.get(("q" if which == "q" else "k", idx))
                            if prev is not None:
                                tensor.wait_ge(s_act, prev)
                            for dt in range(6):
                                mm = tensor.matmul(
                                    qk_ps[idx % 2][:],
                                    wsb[:, dt, jt * 128:(jt + 1) * 128],
                                    xT[:, dt, b * 512:(b + 1) * 512],
                                    start=(dt == 0), stop=(dt == 5),
                                )
                            C["pe"] += 1
                            mm.then_inc(s_pe, 1)
                            qk_done[key] = C["pe"]
                # v matmuls: reuse banks; wait wv loaded + last q/k copies of
                # the banks
                tensor.wait_ge(s_l6, 6 * 16)
                for st in range(8):
                    for jb in range(2):
                        idx = st * 2 + jb
                        if idx < 2:
                            tensor.wait_ge(s_act, act0 + 46 + 2 * idx)
                        else:
                            tensor.wait_ge(s_dve, v_war[idx])
                        for dt in range(6):
                            mm = tensor.matmul(
                                qk_ps[idx % 2][:, 0:384],
                                xT[:, dt, st * 128:(st + 1) * 128],
                                wA[:, dt, jb * 384:(jb + 1) * 384],
                                start=(dt == 0), stop=(dt == 5),
                            )
                        C["pe"] += 1
                        mm.then_inc(s_pe, 1)
                        v_done[(st, jb)] = C["pe"]
                # rel matmuls: need q rows of qaug complete (even-half ACT
                # copies + all 12 q cross DMAs: q crosses are 6 per parity)
                tensor.wait_ge(s_l3, 5 * 16)
                tensor.wait_ge(s_cr0, 6 * 16)
                tensor.wait_ge(s_cr1, 6 * 16)
                tensor.wait_ge(s_act, marks["q_even"])
                qa = qaug[0:64, :, :]
                qa4 = qa.rearrange("p m (h w) -> p m h w", w=32)
                for hq in range(H):
                    if hq >= 1:
                        tensor.wait_ge(s_dve, dve_after_v + 3 * (hq - 1) + 1)
                    mm = tensor.matmul(
                        at_ps[0][0:32, 0:384],
                        relh[:, hq * 32:(hq + 1) * 32],
                        qa[:, :, hq * 32:(hq + 1) * 32],
                        start=True, stop=True,
                    )
                    C["pe"] += 1
                    mm.then_inc(s_pe, 1)
                    rel_done[("h", hq)] = C["pe"]
                    if hq >= 1:
                        tensor.wait_ge(s_dve, dve_after_v + 3 * (hq - 1) + 2)
                    mm2 = tensor.matmul(
                        at_ps[1][0:32, 0:384],
                        relw[:, hq * 32:(hq + 1) * 32],
                        qa4[:, :, :, hq],
                        start=True, stop=True,
                    )
                    C["pe"] += 1
                    mm2.then_inc(s_pe, 1)
                    rel_done[("w", hq)] = C["pe"]

            # WAR bookkeeping filled lazily by the ACT/DVE emitters below;
            # emit PE section AFTER computing those maps in a dry pass.
            # Instead of a dry pass, we exploit the fixed structure:
            #   q/k copies: 2 ACT ops per group, groups in same order as PE
            #   v copies: 1 DVE op per group
            qk_war = {}
            act0 = C["act"]
            for i, which in enumerate(("q", "k")):
                for idx in range(12):
                    g = i * 12 + idx
                    # ACT ops (even, odd) for group g have values act0+2g+1, +2
                    if g >= 2:
                        qk_war[(which, idx)] = act0 + 2 * (g - 2) + 2
            v_war = {}
            dve0 = C["dve"]
            for idx in range(16):
                if idx >= 2:
                    v_war[idx] = dve0 + (idx - 2) + 1
            dve_after_v = dve0 + 16

            # q copies = 24 ACT ops (values act0+1..act0+24)
            marks["q_even"] = act0 + 24

            block.tensor(_pe1)


            # ---- ACT: qk psum copies with bias ----
            def _act1(scalar):
                scalar.wait_ge(s_l3, 5 * 16)  # qkb/projb loaded
                for i, which in enumerate(("q", "k")):
                    dst = qaug if which == "q" else kaug
                    bofs = 0 if which == "q" else 6
                    for jt in range(6):
                        for b in range(2):
                            g = i * 12 + jt * 2 + b
                            scalar.wait_ge(s_pe, qk_done[(which, jt, b)])
                            if g >= 2:
                                # scratch WAR: all same-parity crosses <= g-2
                                scalar.wait_ge([s_cr0, s_cr1][g % 2],
                                               (g // 2) * 16)
                            ps = qk_ps[(jt * 2 + b) % 2]
                            scalar.activation(
                                dst[0:64, 2 * jt, b * 512:(b + 1) * 512],
                                ps[0:64, :],
                                AF.Identity,
                                bias=qkb_sb[0:64, bofs + jt:bofs + jt + 1],
                            ).then_inc(s_act, 1)
                            C["act"] += 1
                            scalar.activation(
                                scr[g % 2][64:128, :],
                                ps[64:128, :],
                                AF.Identity,
                                bias=qkb_sb[64:128, bofs + jt:bofs + jt + 1],
                            ).then_inc(s_act, 1)
                            C["act"] += 1

            block.scalar(_act1)

            # ---- sync: cross-partition hops (odd heads) ----
            def _cross(sync):
                for i, which in enumerate(("q", "k")):
                    dst = qaug if which == "q" else kaug
                    for jt in range(6):
                        for b in range(2):
                            g = i * 12 + jt * 2 + b
                            sync.wait_ge(s_act, act0 + 2 * g + 2)
                            sync.dma_start(
                                out=dst[0:64, 2 * jt + 1, b * 512:(b + 1) * 512],
                                in_=scr[g % 2][64:128, :],
                            ).then_inc([s_cr0, s_cr1][g % 2], 16)

            block.sync(_cross)

            # ---- second sync section: wv loads (into wA after q done) ----
            def _loads2(sync):
                sync.wait_ge(s_pe, qk_done[("q", 5, 1)])  # wA (q weights) free
                for dt in range(6):
                    sync.dma_start(out=wA[:, dt, :],
                                   in_=wv_e[dt * 128:(dt + 1) * 128, :]
                                   ).then_inc(s_l6, 16)

            block.sync(_loads2)

            # ---- DVE: v copies + rel copies ----
            def _dve1(vector):
                va = vaug[:].rearrange("p t (m c) -> p t m c", c=65)
                for st in range(8):
                    for jb in range(2):
                        vector.wait_ge(s_pe, v_done[(st, jb)])
                        src = qk_ps[(st * 2 + jb) % 2][:, 0:384]
                        src3 = src.rearrange("p (m c) -> p m c", c=64)
                        vector.tensor_copy(
                            va[:, st, jb * 6:(jb + 1) * 6, 0:64], src3
                        ).then_inc(s_dve, 1)
                        C["dve"] += 1
                qa_w = qaug[96:128, :, :].rearrange(
                    "p m (h w) -> p m h w", w=32)
                shifted = scr[0][96:128, 0:384].rearrange(
                    "p (m h) -> p m h", h=32)
                for hq in range(H):
                    # c1h: psum -> scr[0] rows 0:32 (all prior rel-h DMAs done)
                    vector.wait_ge(s_pe, rel_done[("h", hq)])
                    if hq >= 1:
                        vector.wait_ge(s_rh, hq * 16)
                    vector.tensor_copy(
                        scr[0][0:32, 0:384], at_ps[0][0:32, 0:384]
                    ).then_inc(s_dve, 1)
                    C["dve"] += 1
                    # c1w: psum -> scr[1] rows 0:32
                    vector.wait_ge(s_pe, rel_done[("w", hq)])
                    if hq >= 1:
                        vector.wait_ge(s_rw, hq * 16)
                    vector.tensor_copy(
                        scr[1][0:32, 0:384], at_ps[1][0:32, 0:384]
                    ).then_inc(s_dve, 1)
                    C["dve"] += 1
                    # c2w: shifted staging -> strided scatter into qaug
                    vector.wait_ge(s_rw, (hq + 1) * 16)
                    vector.tensor_copy(
                        qa_w[:, :, :, hq], shifted
                    ).then_inc(s_dve, 1)
                    C["dve"] += 1

            block.vector(_dve1)

            def _sync_rel(sync):
                qa_h = qaug[64:96, :, :]
                srch = scr[0][0:32, 0:384].rearrange("p (m w) -> p m w", w=32)
                for hq in range(H):
                    # dmah: partition-shift 0:32 -> 64:96 straight into qaug
                    sync.wait_ge(s_dve, dve_after_v + 3 * hq + 1)
                    sync.dma_start(
                        out=qa_h[:, :, hq * 32:(hq + 1) * 32], in_=srch,
                    ).then_inc(s_rh, 16)
                    # shift: scr[1][0:32] -> scr[0][96:128]
                    sync.wait_ge(s_dve, dve_after_v + 3 * hq + 2)
                    if hq == 0:
                        # scr[0][96:128] overlaps the odd-head staging rows;
                        # all 24 q/k cross DMAs must have drained
                        sync.wait_ge(s_cr0, 12 * 16)
                        sync.wait_ge(s_cr1, 12 * 16)
                    else:
                        sync.wait_ge(s_dve, dve_after_v + 3 * (hq - 1) + 3)
                    sync.dma_start(
                        out=scr[0][96:128, 0:384], in_=scr[1][0:32, 0:384],
                    ).then_inc(s_rw, 16)

            block.sync(_sync_rel)

        # phase-1 end marks
        P1 = dict(pe=C["pe"], act=C["act"], dve=C["dve"])

        # ================= PHASE 2: attention =================
        p2 = ExitStack()
        with p2:
            exp_sb = p2.enter_context(
                nc.sbuf_tensor("expp_sb", [128, 16, 512], F32R))
            outdT = p2.enter_context(
                nc.sbuf_tensor("outdT_sb", [128, 6, S], F32R))
            wproj = p2.enter_context(
                nc.sbuf_tensor("wproj_sb", [128, 6, DIM], F32R))
            out_sb = [p2.enter_context(
                nc.sbuf_tensor(f"out_sb{i}", [128, 512], F32)) for i in range(2)]

            # wproj load (overlaps attention)
            def _loadw(sync):
                sync.wait_ge(s_pe, P1["pe"])  # xT/wA regions free
                for dt in range(6):
                    sync.dma_start(out=wproj[:, dt, :],
                                   in_=wproj_e[dt * 128:(dt + 1) * 128, :]
                                   ).then_inc(s_l7, 16)

            block.sync(_loadw)

            # per-iteration sem bases
            PE0, ACT0, DVE0 = C["pe"], C["act"], C["dve"]
            iters = [(m, b) for m in range(NH) for b in range(2)]
            # count of same-slot outdT DMAs before iteration i (slot = i % 2)
            od_before = []
            odc = [0, 0]
            for i, (m, b) in enumerate(iters):
                od_before.append(odc[i % 2])
                if m % 2 == 1:
                    odc[i % 2] += 1

            def pe_base(i):
                return PE0 + 17 * i

            def act_base(i):
                return ACT0 + 11 * i

            def _pe2(tensor):
                # gate on aug tensors fully ready
                tensor.wait_ge(s_act, P1["act"])
                tensor.wait_ge(s_dve, P1["dve"])
                tensor.wait_ge(s_cr0, 12 * 16)
                tensor.wait_ge(s_cr1, 12 * 16)
                tensor.wait_ge(s_rh, H * 16)
                tensor.wait_ge(s_rw, H * 16)
                tensor.wait_ge(s_l4, NH * 16)
                tensor.wait_ge(s_l5, 8 * 16)
                for i, (m, b) in enumerate(iters):
                    pb, ab = pe_base(i), act_base(i)
                    qrhs = qaug[:, m, b * 512:(b + 1) * 512]
                    # order: QK0 QK1 QK2 PV0 QK3 PV1 ... QK7 PV5 PV6 PV7 bcast
                    def qk(t):
                        if i >= 1 and t <= 1:
                            # attn bank WAR vs prev iteration's exp(t+6)
                            tensor.wait_ge(s_act, act_base(i - 1) + 7 + t)
                        if t >= 2:
                            tensor.wait_ge(s_act, ab + (t - 2) + 1)
                        tensor.matmul(
                            at_ps[t % 2][:],
                            kaug[:, m, t * 128:(t + 1) * 128],
                            qrhs,
                            start=True, stop=True,
                        ).then_inc(s_pe, 1)
                        C["pe"] += 1

                    def pv(t):
                        tensor.wait_ge(s_act, ab + t + 1)
                        if t == 0 and i >= 2:
                            # out bank WAR vs iter i-2's DVE mul
                            tensor.wait_ge(s_dve, DVE0 + (i - 2) + 1)
                        tensor.matmul(
                            out_ps[i % 2][0:65, :],
                            vaug[:, t, m * 65:(m + 1) * 65],
                            exp_sb[:, (i % 2) * 8 + t, :],
                            start=(t == 0), stop=(t == 7),
                        ).then_inc(s_pe, 1)
                        C["pe"] += 1

                    qk(0)
                    qk(1)
                    for t in range(6):
                        qk(t + 2)
                        pv(t)
                    pv(6)
                    pv(7)
                    # bcast matmul: wait recip ready (ACT op #10 of iter)
                    tensor.wait_ge(s_act, ab + 10)
                    tensor.matmul(
                        bc_ps[:], ones64[:], recip_sb[:],
                        start=True, stop=True,
                    ).then_inc(s_pe, 1)
                    C["pe"] += 1
                    assert C["pe"] == pb + 17

            block.tensor(_pe2)

            # QK(t) s_pe offsets within iter: QK0=1 QK1=2 QK2=3 QK3=5 QK4=7
            # QK5=9 QK6=11 QK7=13 ; PV in PE_INC_PV
            QK_INC = [1, 2, 3, 5, 7, 9, 11, 13]

            def _act2b(scalar):
                for i, (m, b) in enumerate(iters):
                    pb, ab = pe_base(i), act_base(i)
                    for t in range(8):
                        scalar.wait_ge(s_pe, pb + QK_INC[t])
                        if i >= 2:
                            # exp tile set reuse: PV(t) of iter i-2 done
                            scalar.wait_ge(
                                s_pe, pe_base(i - 2) + PE_INC_PV[t])
                        scalar.activation(
                            exp_sb[:, (i % 2) * 8 + t, :],
                            at_ps[t % 2][:],
                            AF.Exp,
                        ).then_inc(s_act, 1)
                        C["act"] += 1
                    # ln of denominator row (needs PV7)
                    scalar.wait_ge(s_pe, pb + 16)
                    scalar.activation(
                        ln_sb, out_ps[i % 2][64:65, :], AF.Ln,
                    ).then_inc(s_act, 1)
                    C["act"] += 1
                    scalar.activation(
                        recip_sb[:], ln_sb, AF.Exp, scale=-1.0,
                    ).then_inc(s_act, 1)
                    C["act"] += 1
                    # bcast psum -> sbuf copy (needs PE bcast mm; and the
                    # previous iteration's DVE mul must have read bcast_sb)
                    scalar.wait_ge(s_pe, pb + 17)
                    if i >= 1:
                        scalar.wait_ge(s_dve, DVE0 + i)
                    scalar.activation(
                        bcast_sb[:], bc_ps[:], AF.Copy,
                    ).then_inc(s_act, 1)
                    C["act"] += 1
                    assert C["act"] == ab + 11

            block.scalar(_act2b)

            def _dve2(vector):
                for i, (m, b) in enumerate(iters):
                    pb, ab = pe_base(i), act_base(i)
                    vector.wait_ge(s_act, ab + 11)
                    if m % 2 == 0:
                        dst = outdT[0:64, m // 2, b * 512:(b + 1) * 512]
                    else:
                        if i >= 6:
                            # scratch WAR: all prior same-slot outdT DMAs done
                            vector.wait_ge([s_od0, s_od1][i % 2],
                                           (od_before[i] ) * 16)
                        dst = scr[i % 2][0:64, :]
                    vector.tensor_mul(
                        dst, out_ps[i % 2][0:64, :], bcast_sb[:],
                    ).then_inc(s_dve, 1)
                    C["dve"] += 1

            block.vector(_dve2)

            def _sync2(sync):
                for i, (m, b) in enumerate(iters):
                    if m % 2 == 1:
                        sync.wait_ge(s_dve, DVE0 + i + 1)
                        sync.dma_start(
                            out=outdT[64:128, m // 2, b * 512:(b + 1) * 512],
                            in_=scr[i % 2][0:64, :],
                        ).then_inc([s_od0, s_od1][i % 2], 16)

            block.sync(_sync2)

            # ================= PHASE 3: proj =================
            P2 = dict(pe=C["pe"], act=C["act"], dve=C["dve"])
            PRJ_PE0, PRJ_ACT0 = C["pe"], C["act"]

            def _pe3(tensor):
                tensor.wait_ge(s_dve, P2["dve"])
                tensor.wait_ge(s_od0, odc[0] * 16)
                tensor.wait_ge(s_od1, odc[1] * 16)
                tensor.wait_ge(s_l7, 6 * 16)
                for g, (jt, b) in enumerate(
                        [(j, bb) for j in range(6) for bb in range(2)]):
                    if g >= 2:
                        tensor.wait_ge(s_act, PRJ_ACT0 + (g - 2) + 1)
                    for ct in range(6):
                        mm = tensor.matmul(
                            qk_ps[g % 2][:],
                            wproj[:, ct, jt * 128:(jt + 1) * 128],
                            outdT[:, ct, b * 512:(b + 1) * 512],
                            start=(ct == 0), stop=(ct == 5),
                        )
                    mm.then_inc(s_pe, 1)
                    C["pe"] += 1

            block.tensor(_pe3)

            def _act3(scalar):
                for g, (jt, b) in enumerate(
                        [(j, bb) for j in range(6) for bb in range(2)]):
                    scalar.wait_ge(s_pe, PRJ_PE0 + g + 1)
                    if g >= 2:
                        scalar.wait_ge([s_out0, s_out1][g % 2],
                                       (g // 2) * 16)
                    scalar.activation(
                        out_sb[g % 2][:], qk_ps[g % 2][:], AF.Identity,
                        bias=projb_sb[:, jt:jt + 1],
                    ).then_inc(s_act, 1)
                    C["act"] += 1

            block.scalar(_act3)

            def _sync3(sync):
                for g, (jt, b) in enumerate(
                        [(j, bb) for j in range(6) for bb in range(2)]):
                    sync.wait_ge(s_act, PRJ_ACT0 + g + 1)
                    sync.dma_start(
                        out=outT_e[jt * 128:(jt + 1) * 128,
                                   b * 512:(b + 1) * 512],
                        in_=out_sb[g % 2][:],
                    ).then_inc([s_out0, s_out1][g % 2], 16)
                sync.wait_ge(s_out0, 6 * 16)
                sync.wait_ge(s_out1, 6 * 16)

            block.sync(_sync3)

    # clear semaphores so the NEFF is safely re-executable (profiling runs
    # execute it more than once)
    nc.reset()
    return nc


def _prep_inputs(x, qkv_w, qkv_b, proj_w, proj_b, rel_pos_h, rel_pos_w):
    """Host-side constant prep shared across cores (everything but xT)."""
    f32 = np.float32
    wq = qkv_w[0:DIM].astype(f32) * SCALE          # (768, 768) rows j
    wk = qkv_w[DIM:2 * DIM].astype(f32)
    wv = qkv_w[2 * DIM:3 * DIM].astype(f32)
    wqk = np.concatenate([wq.T, wk.T], axis=1).copy()      # (768, 1536) [d, j]
    wv_t = wv.T.copy()                                     # (768, 768)  [d, jv]
    wproj = proj_w.astype(f32).T.copy()                    # (768, 768)  [c, j]

    qb = qkv_b[0:DIM].astype(f32) * SCALE
    kb = qkv_b[DIM:2 * DIM].astype(f32)
    vb = qkv_b[2 * DIM:3 * DIM].astype(f32)
    qkb = np.concatenate(
        [qb.reshape(6, 128).T, kb.reshape(6, 128).T], axis=1).copy()  # (128,12)
    projb_eff = (proj_b.astype(f32) + vb @ proj_w.astype(f32).T)
    projb = projb_eff.reshape(6, 128).T.copy()                        # (128, 6)

    idx = np.arange(H)[:, None] - np.arange(H)[None, :] + (H - 1)
    Rh = rel_pos_h.astype(f32)[idx]            # (32, 32, 64) [hq, kh, c]
    Rw = rel_pos_w.astype(f32)[idx]            # (32, 32, 64) [wq, kw, c]
    # lhsT layout [c, hq*32+k], pre-scaled by 1/SCALE to undo q pre-scaling
    relh = (Rh.transpose(2, 0, 1) / SCALE).reshape(HD, H * H).copy()
    relw = (Rw.transpose(2, 0, 1) / SCALE).reshape(HD, W * W).copy()

    onehot = np.zeros((HD, S), dtype=f32)
    s = np.arange(S)
    onehot[s // W, s] = 1.0          # rows 0:32  = onehot of k_h
    onehot[32 + s % W, s] = 1.0      # rows 32:64 = onehot of k_w
    onescol = np.ones((128, NH), dtype=f32)
    ones64 = np.ones((1, HD), dtype=f32)

    return dict(wqk=wqk, wv=wv_t, wproj=wproj, relh=relh, relw=relw,
                onehot=onehot, onescol=onescol, ones64=ones64,
                qkb=qkb, projb=projb)


_CACHED_NC = None


def kernel(x, qkv_w, qkv_b, proj_w, proj_b, rel_pos_h, rel_pos_w,
           trace=False):
    from concourse.bass_utils import run_bass_kernel_spmd

    global _CACHED_NC
    if _CACHED_NC is None:
        _CACHED_NC = build_nc()
    nc = _CACHED_NC

    consts = _prep_inputs(x, qkv_w, qkv_b, proj_w, proj_b,
                          rel_pos_h, rel_pos_w)
    in_maps = []
    for b in range(NCORES):
        xT = np.ascontiguousarray(
            x[b].reshape(S, DIM).T.astype(np.float32))
        in_maps.append({"xT": xT, **consts})

    res = run_bass_kernel_spmd(nc, in_maps, core_ids=list(range(NCORES)),
                               trace=trace)
    outs = []
    for b in range(NCORES):
        outT = res.results[b]["outT"]          # (768, 1024)
        outs.append(outT.T.reshape(H, W, DIM))
    full = np.stack(outs, axis=0).astype(np.float32)
    if trace:
        return full, res
    return full
